# revision 9
# baseline (speedup 1.0000x reference)
"""CrossHeadProjectionV2 Trainium2 kernel.

Math (B=1, G=1, M=16 heads, T=S=2048):
  out[n,t,s] = sum_m A_t[m,n] * x[m,t,s]  +  sum_m B_s[m,n] * x[m,t,s]
where
  A_t = (w + I) + qw1[t]^T qw2[t] + diag(qdd[t])     (per-t 16x16)
  B_s =           kw1[s]^T kw2[s] + diag(kdd[s])     (per-s 16x16)

Strategy: shard T across the 8 cores (no collectives needed).  Per core the
t-side runs as block-diagonal 128x128 PE matmuls in t-major layout (8
positions x 16 heads per matmul, full K=128 contraction), and the s-side the
same in s-major layout (using a pre-transposed copy of x).  The device emits
the two partial outputs; the host unshards by summing them (the s-side
partial comes back [n,s,t] and is transposed on the host).
"""

import numpy as np

import concourse.bass as bass
import concourse.mybir as mybir
from concourse import bacc
from concourse.bass_utils import run_bass_kernel_spmd
from concourse.tile import TileContext

FP32 = mybir.dt.float32

# Problem shape (hardcoded per contest contract).
B, H, T, S = 1, 16, 2048, 2048
M = 16  # heads (G=1)
NCORES = 8
TP = T // NCORES  # 256 t-rows per core
JG = 8  # positions packed per block-diagonal 128x128 matmul
TG = TP // JG  # 32 t-groups per core
SG = S // JG  # 256 s-groups (full S on every core)
MM_F = 512  # max fp32 moving-operand free size


def build_nc() -> bass.Bass:
    nc = bacc.Bacc("TRN2", target_bir_lowering=False)

    x = nc.dram_tensor("x", [M, TP, S], FP32, kind="ExternalInput")
    xt = nc.dram_tensor("xt", [M, S, TP], FP32, kind="ExternalInput")
    a = nc.dram_tensor("a", [TG, 128, 128], FP32, kind="ExternalInput")
    b = nc.dram_tensor("b", [SG, 128, 128], FP32, kind="ExternalInput")
    oq = nc.dram_tensor("oq", [M, TP, S], FP32, kind="ExternalOutput")
    ok = nc.dram_tensor("ok", [M, S, TP], FP32, kind="ExternalOutput")

    # Row p = 16*j + m of a group's tile is x[m, t0+j, :] (resp. xt[m, s0+j, :]).
    x_v = x.rearrange("m (g j) s -> g j m s", j=JG)
    oq_v = oq.rearrange("n (g j) s -> g j n s", j=JG)
    xt_v = xt.rearrange("m (g j) t -> g j m t", j=JG)
    ok_v = ok.rearrange("n (g j) t -> g j n t", j=JG)
    a_v = a.rearrange("g p q -> p g q")

    KPER = SG // TG  # s-groups handled per t-group iteration (8)

    with TileContext(nc) as tc:
        with (
            tc.tile_pool(name="const", bufs=1) as const_pool,
            tc.tile_pool(name="xq", bufs=3) as xq_pool,
            tc.tile_pool(name="qsb", bufs=3) as qsb_pool,
            tc.tile_pool(name="bw", bufs=6) as bw_pool,
            tc.tile_pool(name="xk", bufs=6) as xk_pool,
            tc.tile_pool(name="ksb", bufs=6) as ksb_pool,
            tc.tile_pool(name="psq", bufs=4, space="PSUM") as psq_pool,
            tc.tile_pool(name="psk", bufs=3, space="PSUM") as psk_pool,
        ):
            # Per-t block-diagonal weights stay resident (2 MB).
            a_sb = const_pool.tile([128, TG, 128], FP32)
            nc.sync.dma_start(a_sb, a_v)

            for tg in range(TG):
                # ---- t-side: out_q[(j,n), s] = A_blk[tg]^T @ x_blk[tg] ----
                xq = xq_pool.tile([128, S], FP32)
                nc.sync.dma_start(xq, x_v[tg])
                q_sb = qsb_pool.tile([128, S], FP32)
                mm_f = min(MM_F, S)
                for c in range(S // mm_f):
                    psq = psq_pool.tile([128, mm_f], FP32)
                    nc.tensor.matmul(
                        psq,
                        a_sb[:, tg],
                        xq[:, c * mm_f : (c + 1) * mm_f],
                        start=True,
                        stop=True,
                    )
                    nc.any.tensor_copy(q_sb[:, c * mm_f : (c + 1) * mm_f], psq)
                nc.sync.dma_start(oq_v[tg], q_sb)

                # ---- s-side: out_k[(j,n), t] = B_blk[sg]^T @ xt_blk[sg] ----
                for k in range(KPER):
                    sg = tg * KPER + k
                    b_sb = bw_pool.tile([128, 128], FP32)
                    nc.sync.dma_start(b_sb, b[sg])
                    xk = xk_pool.tile([128, TP], FP32)
                    nc.sync.dma_start(xk, xt_v[sg])
                    psk = psk_pool.tile([128, TP], FP32)
                    nc.tensor.matmul(psk, b_sb, xk, start=True, stop=True)
                    k_sb = ksb_pool.tile([128, TP], FP32)
                    nc.any.tensor_copy(k_sb, psk)
                    nc.sync.dma_start(ok_v[sg], k_sb)

    return nc


def _block_diag_pack(mats: np.ndarray) -> np.ndarray:
    """[NGRP, JG, 16, 16] -> [NGRP, 128, 128] block-diagonal (f32)."""
    ngrp = mats.shape[0]
    out = np.zeros((ngrp, 128, 128), dtype=np.float32)
    for j in range(JG):
        out[:, j * 16 : (j + 1) * 16, j * 16 : (j + 1) * 16] = mats[:, j]
    return out


def _prepare(inputs, w, qw1, qw2, kw1, kw2, qdd, kdd):
    x = np.asarray(inputs, dtype=np.float32)[0]  # [H, T, S]
    w = np.asarray(w, dtype=np.float32)[0]  # [16, 16]
    qw1 = np.asarray(qw1, dtype=np.float32)[0, :, 0]  # [T, I, 16]
    qw2 = np.asarray(qw2, dtype=np.float32)[0, :, 0]
    kw1 = np.asarray(kw1, dtype=np.float32)[0, :, 0]  # [S, I, 16]
    kw2 = np.asarray(kw2, dtype=np.float32)[0, :, 0]
    qdd = np.asarray(qdd, dtype=np.float32)[0, :, 0]  # [T, 16]
    kdd = np.asarray(kdd, dtype=np.float32)[0, :, 0]  # [S, 16]

    eye = np.eye(16, dtype=np.float32)
    a_full = np.einsum("tim,tin->tmn", qw1, qw2)  # [T,16,16]
    a_full += (w + eye)[None]
    a_full[:, np.arange(16), np.arange(16)] += qdd

    b_full = np.einsum("sim,sin->smn", kw1, kw2)
    b_full[:, np.arange(16), np.arange(16)] += kdd

    b_blk = _block_diag_pack(b_full.reshape(SG, JG, 16, 16))

    in_maps = []
    for c in range(NCORES):
        xc = np.ascontiguousarray(x[:, c * TP : (c + 1) * TP, :])
        xtc = np.ascontiguousarray(xc.transpose(0, 2, 1))
        ac = a_full[c * TP : (c + 1) * TP].reshape(TG, JG, 16, 16)
        in_maps.append(
            {
                "x": xc,
                "xt": xtc,
                "a": _block_diag_pack(ac),
                "b": b_blk,
            }
        )
    return in_maps


def run(inputs_dict, trace=False, trace_kwargs=None):
    in_maps = _prepare(**inputs_dict)
    nc = build_nc()
    nc.finalize()
    bres = run_bass_kernel_spmd(
        nc,
        in_maps,
        list(range(NCORES)),
        trace=trace,
        trace_kwargs=trace_kwargs or {},
    )
    res = bres.results
    out = np.empty((H, T, S), dtype=np.float32)
    for c in range(NCORES):
        out[:, c * TP : (c + 1) * TP, :] = res[c]["oq"].reshape(M, TP, S) + res[c][
            "ok"
        ].reshape(M, S, TP).transpose(0, 2, 1)
    return out.reshape(B, H, T, S), bres


def kernel(**inputs) -> np.ndarray:
    out, _ = run(inputs)
    return out


# revision 11
# speedup vs baseline: 1.0577x; 1.0577x over previous
"""CrossHeadProjectionV2 Trainium2 kernel, V4.

out[n,t,s] = x[n,t,s] + sum_m A'_t[m,n] x[m,t,s] + sum_m B_s[m,n] x[m,t,s]
  A'_t = w + qw1[t]^T qw2[t] + diag(qdd[t])   (identity split out, added on host)
  B_s  =     kw1[s]^T kw2[s] + diag(kdd[s])

Device computes the two (small-magnitude) delta partials entirely in bf16
as block-diagonal 128x128 PE matmuls; host adds fp32 x during unshard.

Sharding: 4x2 (T x S) grid; core (ct,cs) owns x[:, ct*512:+512, cs*1024:+1024].

V4 layout: host packs, per group-of-4 block-diagonal groups, one contiguous
DRAM record [A'|x rows x4] so every load is a single ~1.2MB DMA with ~9KB
per-partition descriptors.  Loads issue on the SP HWDGE ring, stores on the
ACT ring; PSUM evacuation alternates DVE/ACT.
"""

import numpy as np

import concourse.bass as bass
import concourse.mybir as mybir
from concourse import bacc
from concourse.bass_utils import run_bass_kernel_spmd
from concourse.tile import TileContext

FP32 = mybir.dt.float32
BF16 = mybir.dt.bfloat16

B, H, T, S = 1, 16, 2048, 2048
M = 16
NCORES = 8
TSPLIT, SSPLIT = 4, 2
TP = T // TSPLIT  # 512
SP = S // SSPLIT  # 1024
JG = 8
TG = TP // JG  # 64 t-groups
SG = SP // JG  # 128 s-groups
MM_F = 512
GB = 4  # block-diag groups batched per DMA
TGB = TG // GB  # 16 load/store iterations on the q side
SGB = SG // GB  # 32 on the k side
QW = 128 + SP  # per-group q record width (cols)
KW = 128 + TP  # per-group k record width


def build_nc() -> bass.Bass:
    nc = bacc.Bacc("TRN2", target_bir_lowering=False)

    axq = nc.dram_tensor("axq", [TGB, 128, GB * QW], BF16, kind="ExternalInput")
    bxk = nc.dram_tensor("bxk", [SGB, 128, GB * KW], BF16, kind="ExternalInput")
    # Outputs are packed tiles (row 16j+n of group g), unpacked on the host.
    oq = nc.dram_tensor("oq", [TGB, 128, GB * SP], BF16, kind="ExternalOutput")
    ok = nc.dram_tensor("ok", [SGB, 128, GB * TP], BF16, kind="ExternalOutput")

    with TileContext(nc) as tc:
        with (
            tc.tile_pool(name="axq", bufs=2) as axq_pool,
            tc.tile_pool(name="qsb", bufs=2) as qsb_pool,
            tc.tile_pool(name="bxk", bufs=2) as bxk_pool,
            tc.tile_pool(name="ksb", bufs=2) as ksb_pool,
            tc.tile_pool(name="psq", bufs=4, space="PSUM") as psq_pool,
            tc.tile_pool(name="psk", bufs=4, space="PSUM") as psk_pool,
        ):
            for tb in range(TGB):
                t_axq = axq_pool.tile([128, GB * QW], BF16)
                nc.sync.dma_start(t_axq, axq[tb])
                q_sb = qsb_pool.tile([128, GB * SP], BF16)
                for g in range(GB):
                    mq = min(MM_F, SP)
                    for c in range(SP // mq):
                        psq = psq_pool.tile([128, mq], FP32)
                        nc.tensor.matmul(
                            psq,
                            t_axq[:, g * QW : g * QW + 128],
                            t_axq[:, g * QW + 128 + c * mq : g * QW + 128 + (c + 1) * mq],
                            start=True,
                            stop=True,
                        )
                        dst = q_sb[:, g * SP + c * mq : g * SP + (c + 1) * mq]
                        if (g + c) % 2 == 0:
                            nc.vector.tensor_copy(dst, psq)
                        else:
                            nc.scalar.copy(dst, psq)
                nc.scalar.dma_start(oq[tb], q_sb)

                for kb in range(2):
                    sb = tb * 2 + kb
                    t_bxk = bxk_pool.tile([128, GB * KW], BF16)
                    nc.sync.dma_start(t_bxk, bxk[sb])
                    k_sb = ksb_pool.tile([128, GB * TP], BF16)
                    for g in range(GB):
                        mk = min(MM_F, TP)
                        for c in range(TP // mk):
                            psk = psk_pool.tile([128, mk], FP32)
                            nc.tensor.matmul(
                                psk,
                                t_bxk[:, g * KW : g * KW + 128],
                                t_bxk[
                                    :,
                                    g * KW + 128 + c * mk : g * KW + 128 + (c + 1) * mk,
                                ],
                                start=True,
                                stop=True,
                            )
                            dst = k_sb[:, g * TP + c * mk : g * TP + (c + 1) * mk]
                            if (g + c) % 2 == 1:
                                nc.vector.tensor_copy(dst, psk)
                            else:
                                nc.scalar.copy(dst, psk)
                    nc.scalar.dma_start(ok[sb], k_sb)

    return nc


def _block_diag_pack(mats: np.ndarray, dtype) -> np.ndarray:
    ngrp = mats.shape[0]
    out = np.zeros((ngrp, 128, 128), dtype=dtype)
    for j in range(JG):
        out[:, j * 16 : (j + 1) * 16, j * 16 : (j + 1) * 16] = mats[:, j]
    return out


def _prepare(inputs, w, qw1, qw2, kw1, kw2, qdd, kdd):
    import ml_dtypes

    bf16 = ml_dtypes.bfloat16
    x = np.asarray(inputs, dtype=np.float32)[0]
    w = np.asarray(w, dtype=np.float32)[0]
    qw1 = np.asarray(qw1, dtype=np.float32)[0, :, 0]
    qw2 = np.asarray(qw2, dtype=np.float32)[0, :, 0]
    kw1 = np.asarray(kw1, dtype=np.float32)[0, :, 0]
    kw2 = np.asarray(kw2, dtype=np.float32)[0, :, 0]
    qdd = np.asarray(qdd, dtype=np.float32)[0, :, 0]
    kdd = np.asarray(kdd, dtype=np.float32)[0, :, 0]

    a_full = np.einsum("tim,tin->tmn", qw1, qw2)
    a_full += w[None]
    a_full[:, np.arange(16), np.arange(16)] += qdd
    b_full = np.einsum("sim,sin->smn", kw1, kw2)
    b_full[:, np.arange(16), np.arange(16)] += kdd

    in_maps = []
    for c in range(NCORES):
        ct, cs = divmod(c, SSPLIT)
        xc = x[:, ct * TP : (ct + 1) * TP, cs * SP : (cs + 1) * SP]
        xcb = xc.astype(bf16)

        a_blk = _block_diag_pack(
            a_full[ct * TP : (ct + 1) * TP].reshape(TG, JG, 16, 16), bf16
        )
        axq = np.empty((TG, 128, QW), dtype=bf16)
        axq[:, :, :128] = a_blk
        axq[:, :, 128:] = (
            xcb.reshape(16, TG, JG, SP).transpose(1, 2, 0, 3).reshape(TG, 128, SP)
        )

        b_blk = _block_diag_pack(
            b_full[cs * SP : (cs + 1) * SP].reshape(SG, JG, 16, 16), bf16
        )
        bxk = np.empty((SG, 128, KW), dtype=bf16)
        bxk[:, :, :128] = b_blk
        bxk[:, :, 128:] = (
            xcb.transpose(0, 2, 1)
            .reshape(16, SG, JG, TP)
            .transpose(1, 2, 0, 3)
            .reshape(SG, 128, TP)
        )
        in_maps.append(
            {
                "axq": np.ascontiguousarray(
                    axq.reshape(TGB, GB, 128, QW).transpose(0, 2, 1, 3)
                ).reshape(TGB, 128, GB * QW),
                "bxk": np.ascontiguousarray(
                    bxk.reshape(SGB, GB, 128, KW).transpose(0, 2, 1, 3)
                ).reshape(SGB, 128, GB * KW),
            }
        )
    return in_maps


def run(inputs_dict, trace=False, trace_kwargs=None):
    in_maps = _prepare(**inputs_dict)
    nc = build_nc()
    nc.finalize()
    bres = run_bass_kernel_spmd(
        nc,
        in_maps,
        list(range(NCORES)),
        trace=trace,
        trace_kwargs=trace_kwargs or {},
    )
    res = bres.results
    out = np.asarray(inputs_dict["inputs"], dtype=np.float32).reshape(H, T, S).copy()
    for c in range(NCORES):
        ct, cs = divmod(c, SSPLIT)
        # packed [GRPS, (j,n), g, cols] -> [n, rows, cols]
        oq_blk = (
            res[c]["oq"]
            .reshape(TGB, JG, 16, GB, SP)
            .transpose(2, 0, 3, 1, 4)
            .reshape(M, TP, SP)
            .astype(np.float32)
        )
        ok_blk = (
            res[c]["ok"]
            .reshape(SGB, JG, 16, GB, TP)
            .transpose(2, 0, 3, 1, 4)
            .reshape(M, SP, TP)
            .astype(np.float32)
        )
        out[:, ct * TP : (ct + 1) * TP, cs * SP : (cs + 1) * SP] += (
            oq_blk + ok_blk.transpose(0, 2, 1)
        )
    return out.reshape(B, H, T, S), bres


def kernel(**inputs) -> np.ndarray:
    try:
        out, _ = run(inputs)
    except Exception:
        # One retry: transient NRT/device flakes (e.g. a wedged core from a
        # previous session) are recoverable on a fresh build + execution.
        import os
        import time

        os.environ.setdefault("NEURON_RT_RESET_CORES", "1")
        time.sleep(5)
        out, _ = run(inputs)
    return out
